# revision 36
# baseline (speedup 1.0000x reference)
"""CRR binomial-tree American put pricer on Trainium2 (Bass/Tile), v2.

Math per element (faithful to the reference):
  dt = T/n; u = exp(sigma*sqrt(dt)); d = 1/u
  p = clip((exp(R*dt) - d) / (u - d + 1e-8), 0, 1); disc = exp(-R*dt)
  terminal V[j] = max(K - S*u^j*d^(n-j), 0),  j = 0..n
  backward:  V[j] = max(disc*(p*V[j+1] + (1-p)*V[j]), K - S_s[j])

Device formulation. With a = disc*p, b = disc*(1-p), r = p/(1-p), the
step is V = max(b*(r*V_up + V), E). The b-multiply is absorbed by
running chunks of C steps in a scaled domain (factor b^-(t+1) at local
step t), so the combine is just two wide fused tensor_tensor ops over
all batch groups at once:
    t3 = V[j+1] * r            (r broadcast [P,G,1])
    V  = t3 + V                (in-place aligned)
    V  = max(V, E~)            only on the exercise band [0, We(s))
    ...every C steps: V *= b^C (returns to the true domain)
Scaled exercise E~_s[j] = (K - u^(n-s)*S_n[j]) * b^-(t+1)
              = K*b^-(t+1) + (u^(n-s)*b^-(t+1)) * (-S_n[j])
is produced per group by the otherwise-idle ACT engine as
Identity(scale*SpN + bias) with SpN = -S_n kept resident, and
scale/bias read from small per-chunk ACT-built Exp tables. Above the
band E <= 0, so skipping the max is exact; beyond cap = (n+cmax)/2 + 2
the payoff and value are identically zero, so all ops are truncated
there.

Batch layout: elements sorted by moneyness c = ln(K/S)/ln(u) and
interleaved so each core holds the same 32 quantile groups of 128
lanes. Groups 0..G-2 run fused ("span A", shared width cap); the top
group (largest c, width up to n+1) runs per-group stt ops ("span B").

Why: the wall metric carries a ~90ms fixed dispatch floor, and v1
burned ~50ms of device time in ~40k tiny per-group instructions with
ACT<->DVE ping-pong. v2 runs ~2.8k DVE + ~17k ACT instructions with
~12ms of predicted device time. All inputs ship as ONE packed DRAM
tensor (per-array transfer overhead is ~1ms on the axon path).

Sharding: pure data parallel, batch 32768 -> 8 cores x 4096.
"""

import numpy as np

N_STEPS = 512
RATE = 0.03
N_CORES = 8
P = 128
MARGIN = 16

ROWS = ["r", "b", "x", "beta", "lnKb", "lnSd", "K"]

_cache = {}


def _host_constants(S, K, sigma, T):
    """Per-element scalar constants, computed in float64 then cast."""
    S64 = S.astype(np.float64)
    K64 = K.astype(np.float64)
    sig = sigma.astype(np.float64)
    T64 = T.astype(np.float64)
    n = N_STEPS
    dt = T64 / n
    x = sig * np.sqrt(dt)                # ln u
    u = np.exp(x)
    d = 1.0 / u
    exp_rdt = np.exp(RATE * dt)
    p = np.clip((exp_rdt - d) / (u - d + 1e-8), 0.0, 1.0)
    disc = np.exp(-RATE * dt)
    b = disc * (1.0 - p)
    r = p / (1.0 - p)
    beta = -np.log(b)
    lnSd = np.log(S64) - n * x           # ln(S * d^n)
    f32 = lambda a: np.ascontiguousarray(a, dtype=np.float32)
    consts = dict(r=f32(r), b=f32(b), x=f32(x), beta=f32(beta),
                  lnKb=f32(np.log(K64) + beta), lnSd=f32(lnSd), K=f32(K64))
    c = np.log(K64 / S64) / x            # node j at level s is ITM iff 2j-s < c
    return consts, c, beta, x


def _pick_C(max_beta, max_x):
    """Largest chunk size whose in-chunk scale growth stays well inside
    fp32 range (exponent budget 80, spow needs n*x + C*beta)."""
    budget = 80.0 - N_STEPS * max_x
    for C in (64, 32, 16, 8):
        if N_STEPS % C == 0 and C * max_beta <= max(10.0, budget):
            return C
    return 8


def _schedule(c_sorted, groups, lanes, pmax):
    """Span-A shared width cap, per-step exercise-band tops We, and the
    span-A combine width WtA: truncated above at the exercise band and
    at pmax*s + 6 sigma sqrt(s) (binomial paths beyond carry ~1e-8
    weight; the truncation-edge column feeds in with that weight)."""
    n = N_STEPS
    cmax = [float(c_sorted[(g + 1) * lanes - 1]) for g in range(groups)]
    capA = int(np.clip(np.floor((n + cmax[max(groups - 2, 0)]) / 2) + 2,
                       2, n + 1))
    WeA = np.zeros(n, np.int32)
    WeB = np.zeros(n, np.int32)
    WtA = np.zeros(n, np.int32)
    cA = cmax[max(groups - 2, 0)]
    cB = cmax[groups - 1]
    pq = max(pmax * (1.0 - pmax), 0.1)
    for s in range(n):
        WA = min(s + 1, capA - 1)
        WeA[s] = int(np.clip(np.ceil((s + cA) / 2) + MARGIN, 1, WA))
        utr = int(np.ceil(pmax * s + 6.0 * np.sqrt(s * pq))) + 2
        WtA[s] = int(np.clip(max(WeA[s] + 8, utr), 1, WA))
        WB = min(s + 1, n)
        WeB[s] = int(np.clip(np.ceil((s + cB) / 2) + MARGIN, 1, WB))
    return capA, WeA, WeB, WtA


def _core_perm(order, batch):
    """Interleave the sorted order so that every core's group g covers
    the same moneyness quantile."""
    q = np.arange(batch)
    ggroup = q // (N_CORES * P)
    core = (q % (N_CORES * P)) // P
    lane = q % P
    slot = ggroup * P + lane
    perm = np.empty((N_CORES, batch // N_CORES), dtype=np.int64)
    perm[core, slot] = order
    return perm


def _build(groups, capA, WeA, WeB, WtA, C, _mode="full"):
    """_mode: benchmarking knob — "full" (real kernel), "noe" (skip ACT
    exercise ops), "nomax" (European: skip exercise+max), "gpsmax" (max
    on GPSIMD)."""
    import concourse.bacc as bacc
    import concourse.tile as tile
    from concourse import mybir

    n = N_STEPS
    W0 = n + 1
    GA = groups - 1
    gB = groups - 1
    BC = groups * P
    f32 = mybir.dt.float32
    Alu = mybir.AluOpType
    Act = mybir.ActivationFunctionType
    n_chunks = n // C
    WeAmax = int(max(WeA))
    WeBmax = int(max(WeB))

    LPP = 8                                      # tree levels per pass
    n_pairs = C // LPP
    W0G = W0 + 1                                 # +1 zero guard column

    nc = bacc.Bacc("TRN2", target_bir_lowering=False, debug=False)
    cst_d = nc.dram_tensor("cst", [len(ROWS) * BC], f32,
                           kind="ExternalInput")
    out_d = nc.dram_tensor("out", [BC], f32, kind="ExternalOutput")

    with tile.TileContext(nc) as tc:
        with (
            tc.tile_pool(name="state", bufs=1) as st,
            tc.tile_pool(name="tmp", bufs=1) as tp,
        ):
            VA = st.tile([P, GA, capA + 1], f32)  # +1 zero guard col
            VB = st.tile([P, W0G], f32)
            SpA = st.tile([P, GA, capA], f32)    # -S_n, span A
            SpB = st.tile([P, W0G], f32)         # -S_n, span B
            t3 = st.tile([P, GA, capA + 1], f32)  # +1 guard col
            cstt = st.tile([P, len(ROWS), groups], f32)
            jb = st.tile([P, W0G], f32)
            jbi = st.tile([P, W0G], mybir.dt.int32)
            price = st.tile([P, groups], f32)
            kpow = st.tile([P, GA, n_pairs], f32)  # K * b^-2(p+1)
            rA = st.tile([P, GA], f32)
            wA = st.tile([P, groups], f32)       # 2x = ln(u^2)
            b2 = st.tile([P, groups], f32)       # b^2
            sgx2 = st.tile([P, groups], f32)     # 2(x + beta)
            bCt = st.tile([P, GA], f32)          # b^C
            biasA = st.tile([P, n_chunks, groups], f32)
            biasB = st.tile([P, n_chunks, groups], f32)

            def row(name):
                return cstt[:, ROWS.index(name), :]

            def rsc(name, g):
                return cstt[:, ROWS.index(name), g:g + 1]

            nc.sync.dma_start(
                out=cstt,
                in_=cst_d[:].rearrange("(p n g) -> p n g", p=P,
                                       n=len(ROWS)))
            nc.gpsimd.iota(jbi, pattern=[[1, W0G]], base=0,
                           channel_multiplier=0)
            nc.vector.tensor_copy(jb, jbi)

            # derived per-element constants (L = LPP levels per pass)
            bt2 = st.tile([P, groups], f32)      # 2 beta
            btL = st.tile([P, groups], f32)      # L beta
            sgxL = st.tile([P, groups], f32)     # L (x + beta)
            lnKbL = st.tile([P, groups], f32)    # lnK + L beta
            bL = st.tile([P, groups], f32)       # b^L
            wL = st.tile([P, groups], f32)       # L x
            nc.vector.tensor_copy(rA, row("r")[:, 0:GA])
            nc.vector.tensor_tensor(out=wA, in0=row("x"), in1=row("x"),
                                    op=Alu.add)
            nc.vector.tensor_tensor(out=b2, in0=row("b"), in1=row("b"),
                                    op=Alu.mult)
            nc.vector.tensor_tensor(out=bt2, in0=row("beta"),
                                    in1=row("beta"), op=Alu.add)
            nc.vector.tensor_tensor(out=sgx2, in0=wA, in1=bt2, op=Alu.add)
            nc.vector.tensor_tensor(out=btL, in0=bt2, in1=bt2, op=Alu.add)
            nc.vector.tensor_tensor(out=sgxL, in0=sgx2, in1=sgx2, op=Alu.add)
            nc.vector.tensor_tensor(out=bL, in0=b2, in1=b2, op=Alu.mult)
            nc.vector.tensor_tensor(out=wL, in0=wA, in1=wA, op=Alu.add)
            for _ in range(LPP // 8):            # L=8: one more doubling
                nc.vector.tensor_tensor(out=btL, in0=btL, in1=btL,
                                        op=Alu.add)
                nc.vector.tensor_tensor(out=sgxL, in0=sgxL, in1=sgxL,
                                        op=Alu.add)
                nc.vector.tensor_tensor(out=bL, in0=bL, in1=bL, op=Alu.mult)
                nc.vector.tensor_tensor(out=wL, in0=wL, in1=wL, op=Alu.add)
            nc.vector.scalar_tensor_tensor(
                out=lnKbL, in0=row("beta"), scalar=float(LPP - 1),
                in1=row("lnKb"), op0=Alu.mult, op1=Alu.add)
            nc.scalar.activation(bCt, row("beta")[:, 0:GA], Act.Exp,
                                 bias=0.0, scale=-float(C))
            for k in range(n_chunks):
                # biasA = (kC+L)x + L beta ; biasB = (kC+L)x
                nc.vector.scalar_tensor_tensor(
                    out=biasA[:, k, :], in0=row("x"),
                    scalar=float(k * C + LPP),
                    in1=btL, op0=Alu.mult, op1=Alu.add)
                nc.vector.tensor_scalar(
                    out=biasB[:, k, :], in0=row("x"),
                    scalar1=float(k * C + LPP), scalar2=None, op0=Alu.mult)

            # kpow[p] = exp(lnK + L(p+1) beta), shared by all chunks
            for g in range(GA):
                nc.scalar.activation(kpow[:, g, :], jb[:, 0:n_pairs], Act.Exp,
                                     bias=lnKbL[:, g:g + 1],
                                     scale=btL[:, g:g + 1])
            nc.vector.memset(VA, 0.0)
            nc.vector.memset(t3, 0.0)

            # terminal: SpN = -exp(lnSd + 2x*j); V = relu(K + SpN)
            for g in range(GA):
                nc.scalar.activation(t3[:, g, 0:capA], jb[:, 0:capA], Act.Exp,
                                     bias=rsc("lnSd", g),
                                     scale=wA[:, g:g + 1])
            nc.scalar.activation(SpA, t3[:, :, 0:capA], Act.Identity,
                                 bias=0.0, scale=-1.0)
            for g in range(GA):
                nc.scalar.activation(VA[:, g, 0:capA], SpA[:, g, :], Act.Relu,
                                     bias=rsc("K", g), scale=1.0)
            nc.scalar.activation(VB, jb, Act.Exp, bias=rsc("lnSd", gB),
                                 scale=wA[:, gB:gB + 1])
            nc.scalar.activation(SpB, VB, Act.Identity, bias=0.0, scale=-1.0)
            nc.scalar.activation(VB, SpB, Act.Relu, bias=rsc("K", gB),
                                 scale=1.0)

            rB3 = rA[:, :, None]
            bC3 = bCt[:, :, None]
            for k in range(n_chunks):
                s0 = n - 1 - k * C
                # pair tables: spow[p] = exp((n-s_lo)x + 2(p+1)beta) with
                # s_lo = s0-1-2p, mpB[p] = exp((n-s_lo)x)
                spow = tp.tile([P, GA, n_pairs], f32, tag="spow", bufs=2)
                mpB = tp.tile([P, n_pairs], f32, tag="mpB", bufs=2)
                for g in range(GA):
                    nc.scalar.activation(spow[:, g, :], jb[:, 0:n_pairs],
                                         Act.Exp,
                                         bias=biasA[:, k, g:g + 1],
                                         scale=sgxL[:, g:g + 1])
                nc.scalar.activation(mpB, jb[:, 0:n_pairs], Act.Exp,
                                     bias=biasB[:, k, gB:gB + 1],
                                     scale=wL[:, gB:gB + 1])
                for p in range(n_pairs):
                    s_hi = s0 - LPP * p
                    s_lo = s_hi - (LPP - 1)
                    W1 = int(WtA[s_hi])
                    Wp = min(W1 + 1, capA)
                    # descending pass widths: pass i reads only what pass
                    # i-1 wrote (t3 is never rescaled, so a read past the
                    # fresh region picks up a prior chunk's huge-scale
                    # value and diverges)
                    W2 = min(int(WtA[s_lo]), Wp - (LPP - 1))
                    W2 = max(W2, 1)
                    weA = min(int(WeA[s_lo]), W2)
                    do_e = _mode in ("full", "gpsmax")
                    # ACT: scaled exercise values at the pass's low level
                    e3 = tp.tile([P, GA, WeAmax], f32, tag="E3A", bufs=1)
                    if do_e:
                        for g in range(GA):
                            nc.scalar.activation(
                                e3[:, g, 0:weA], SpA[:, g, 0:weA],
                                Act.Identity, bias=kpow[:, g, p:p + 1],
                                scale=spow[:, g, p:p + 1])
                    e3b = tp.tile([P, WeBmax], f32, tag="E3B", bufs=2)
                    weB = min(int(WeB[s_lo]), s_hi + 1 - (LPP - 1))
                    if do_e:
                        nc.scalar.activation(e3b[:, 0:weB], SpB[:, 0:weB],
                                             Act.Identity, bias=rsc("K", gB),
                                             scale=mpB[:, p:p + 1])
                    # DVE span A, LPP levels per pass, V/t3 ping-pong
                    for i in range(LPP):
                        src, dst = (VA, t3) if i % 2 == 0 else (t3, VA)
                        Wi = (min(Wp - i, s_hi + 1 - i) if i < LPP - 1
                              else W2)
                        Wi = max(Wi, 1)
                        shI = [P, GA, Wi]
                        nc.vector.tensor_tensor(
                            out=dst[:, :, 0:Wi], in0=src[:, :, 1:Wi + 1],
                            in1=rB3.to_broadcast(shI), op=Alu.mult)
                        nc.vector.tensor_tensor(
                            out=dst[:, :, 0:Wi], in0=dst[:, :, 0:Wi],
                            in1=src[:, :, 0:Wi], op=Alu.add)
                    if _mode != "nomax":
                        e_src = e3 if do_e else SpA
                        nc.vector.tensor_tensor(
                            out=VA[:, :, 0:weA], in0=VA[:, :, 0:weA],
                            in1=e_src[:, :, 0:weA], op=Alu.max)
                    # DVE span B (unscaled): LPP stt combines + b^L max
                    t3b = tp.tile([P, W0G], f32, tag="T3B", bufs=2)
                    t3b2 = tp.tile([P, W0G], f32, tag="T3B2", bufs=2)
                    for i in range(LPP):
                        srcB = VB if i == 0 else (t3b if i % 2 == 1 else t3b2)
                        dstB = t3b if i % 2 == 0 else t3b2
                        WiB = min(s_hi + 1, n) - i
                        nc.vector.scalar_tensor_tensor(
                            out=dstB[:, 0:WiB], in0=srcB[:, 1:WiB + 1],
                            scalar=rsc("r", gB), in1=srcB[:, 0:WiB],
                            op0=Alu.mult, op1=Alu.add)
                    lastB = t3b if LPP % 2 == 1 else t3b2
                    W2B = min(s_hi + 1, n) - (LPP - 1)
                    if _mode == "nomax":
                        nc.vector.tensor_scalar(
                            out=VB[:, 0:W2B], in0=lastB[:, 0:W2B],
                            scalar1=bL[:, gB:gB + 1], scalar2=None,
                            op0=Alu.mult)
                    else:
                        eb_src = e3b if do_e else SpB
                        nc.vector.scalar_tensor_tensor(
                            out=VB[:, 0:weB], in0=lastB[:, 0:weB],
                            scalar=bL[:, gB:gB + 1], in1=eb_src[:, 0:weB],
                            op0=Alu.mult, op1=Alu.max)
                        if weB < W2B:
                            nc.vector.tensor_scalar(
                                out=VB[:, weB:W2B], in0=lastB[:, weB:W2B],
                                scalar1=bL[:, gB:gB + 1], scalar2=None,
                                op0=Alu.mult)
                # end of chunk: rescale span A back to the true domain,
                # covering every column written this chunk
                s_end = s0 - C + 1
                if k < n_chunks - 1:
                    WR = min(capA, int(WtA[s0]) + 1)
                    shR = [P, GA, WR]
                    nc.vector.tensor_tensor(
                        out=VA[:, :, 0:WR], in0=VA[:, :, 0:WR],
                        in1=bC3.to_broadcast(shR), op=Alu.mult)

            # price: span A still carries b^-C after the last chunk
            nc.vector.tensor_tensor(out=price[:, 0:GA], in0=VA[:, :, 0],
                                    in1=bCt, op=Alu.mult)
            nc.vector.tensor_copy(price[:, GA:groups], VB[:, 0:1])
            nc.sync.dma_start(
                out=out_d[:].rearrange("(g p) -> p g", p=P), in_=price)

    nc.compile()
    return nc


def _in_maps(consts, perm):
    maps = []
    for cidx in range(N_CORES):
        sel = perm[cidx]
        cst = np.stack([consts[name][sel] for name in ROWS])  # [NR, g*P]
        groups = cst.shape[1] // P
        # partition-major layout (p n g) so the DMA bursts contiguously
        cst = cst.reshape(len(ROWS), groups, P).transpose(2, 0, 1)
        maps.append(
            {"cst": np.ascontiguousarray(cst.reshape(-1), dtype=np.float32)})
    return maps


def _prepare(S, K, sigma, T):
    batch = S.shape[0]
    assert batch % (N_CORES * P) == 0, batch
    groups = batch // N_CORES // P
    consts, c, beta, x = _host_constants(S, K, sigma, T)
    order = np.argsort(c)
    perm = _core_perm(order, batch)
    pmax = float((consts["r"] / (1.0 + consts["r"])).max())
    capA, WeA, WeB, WtA = _schedule(c[order], groups, N_CORES * P, pmax)
    C = _pick_C(float(beta.max()), float(x.max()))
    key = (groups, capA, C, WeA.tobytes(), WeB.tobytes(), WtA.tobytes())
    return consts, perm, key, (groups, capA, WeA, WeB, WtA, C)


_exec_cache = {}


def _make_executor(nc):
    """jit-compiled 8-core runner for nc, memoized so repeated kernel()
    calls don't re-trace (run_bass_via_pjrt builds a fresh closure per
    call)."""
    import jax
    from jax.sharding import Mesh, PartitionSpec
    from jax.experimental.shard_map import shard_map
    from concourse import mybir
    from concourse.bass2jax import (
        _bass_exec_p, install_neuronx_cc_hook, partition_id_tensor)

    install_neuronx_cc_hook()
    partition_name = (nc.partition_id_tensor.name
                      if nc.partition_id_tensor else None)
    in_names, out_names, out_avals, zero_outs = [], [], [], []
    for alloc in nc.m.functions[0].allocations:
        if not isinstance(alloc, mybir.MemoryLocationSet):
            continue
        name = alloc.memorylocations[0].name
        if alloc.kind == "ExternalInput":
            if name != partition_name:
                in_names.append(name)
        elif alloc.kind == "ExternalOutput":
            shape = tuple(alloc.tensor_shape)
            dtype = mybir.dt.np(alloc.dtype)
            out_avals.append(jax.core.ShapedArray(shape, dtype))
            zero_outs.append(np.zeros(shape, dtype))
            out_names.append(name)
    n_params = len(in_names)
    n_outs = len(out_avals)
    all_in = in_names + out_names
    if partition_name is not None:
        all_in.append(partition_name)
    donate = tuple(range(n_params, n_params + n_outs))

    def _body(*args):
        operands = list(args)
        if partition_name is not None:
            operands.append(partition_id_tensor())
        outs = _bass_exec_p.bind(
            *operands,
            out_avals=tuple(out_avals),
            in_names=tuple(all_in),
            out_names=tuple(out_names),
            lowering_input_output_aliases=(),
            sim_require_finite=True,
            sim_require_nnan=True,
            nc=nc,
        )
        return tuple(outs)

    devices = jax.devices()[:N_CORES]
    mesh = Mesh(np.asarray(devices), ("core",))
    sharded = jax.jit(
        shard_map(
            _body, mesh=mesh,
            in_specs=(PartitionSpec("core"),) * (n_params + n_outs),
            out_specs=(PartitionSpec("core"),) * n_outs,
            check_rep=False,
        ),
        donate_argnums=donate, keep_unused=True,
    )

    def run(maps):
        concat_in = [
            np.concatenate([np.asarray(maps[c][n]) for c in range(N_CORES)])
            for n in in_names]
        zeros = [np.zeros((N_CORES * z.shape[0], *z.shape[1:]), z.dtype)
                 for z in zero_outs]
        out = sharded(*concat_in, *zeros)
        return np.asarray(out[0]).reshape(N_CORES, -1)

    return run


def kernel(S, K, sigma, T):
    consts, perm, key, args = _prepare(S, K, sigma, T)
    if key not in _cache:
        _cache[key] = _build(*args)
    nc = _cache[key]
    if key not in _exec_cache:
        _exec_cache[key] = _make_executor(nc)
    maps = _in_maps(consts, perm)
    for attempt in range(3):
        try:
            res = _exec_cache[key](maps)
            break
        except Exception:
            # The first execution after another process released the
            # device occasionally hits a transient
            # NRT_EXEC_UNIT_UNRECOVERABLE; the failure resets the
            # device, so re-init the backend and retry.
            if attempt == 2:
                raise
            _exec_cache.clear()
            try:
                import jax
                import jax.extend.backend as jeb
                jax.clear_caches()
                jeb.clear_backends()
            except Exception:
                pass
            _exec_cache[key] = _make_executor(nc)

    batch = S.shape[0]
    out = np.empty(batch, dtype=np.float32)
    for core in range(N_CORES):
        out[perm[core]] = res[core]
    return out
